# revision 20
# baseline (speedup 1.0000x reference)
"""GroupQueryAttention Bass kernel for Trainium2 (8 NeuronCores).

Problem: B=4, S=2048, E=1024, 16 Q-heads, 4 KV-heads (groups), head_dim=64.
Reference quirk: group g attends with K/V "head" g (of the 4 HPG slots), and the
output is flattened in (p, g, d) order: out channel = p*256 + g*64 + d.

Sharding: 8 cores = 4 batches x 2 sequence halves. Each core receives the full
x[b] (rows reordered so its own query half comes first -- attention is invariant
to key/value ordering) and computes a complete [1024, 1024] slice of the output.
No cross-core communication needed; the host concatenates slices.

Per-core dataflow (all fp32):
  1. Two DMA queues in parallel: x tiles stream on the SP queue while all
     weights stream on the Activation DGE queue, so projection matmuls are
     never head-of-line blocked behind the x stream.
  2. PE-transpose x -> xT [e, s]; QT = Wq^T x^T (+bq), KT (dup'd per group for
     row-packed QK), V_ext = x Wv with a ones column per head (softmax
     denominator rides the PV matmul for free). PSUM->SBUF copies alternate
     between DVE and the otherwise-idle ScalarE.
  3. Attention is a ScalarE-exp-bound pipeline (~1.1us per [128,1024] tile, 256
     tiles/core): score tiles rotate through a 2-buffer PSUM pool, one tile per
     (k-tile, head) half-step, so exp streams back-to-back while the PE runs QK
     one half-step ahead and PV one k-tile behind -- including across pair
     boundaries (the first QK of pair i+1 is emitted before the last PV of
     pair i). exp folds in scale=1/8; no max-subtract needed (scores/8 ~
     N(0,1), fp32-safe).
  4. Per-pair epilogue on otherwise-idle engines (DVE/GpSimd/DMA): copy
     unnormalized PV + denominator out of PSUM (releasing the accumulator
     banks), reciprocal, GpSimd partition-broadcast of 1/den, in-place aligned
     normalize. The last pair instead broadcasts via a PE ones-outer-product
     (PSUM is free by then) to shorten the serial tail before the
     O-projection (+bo).
"""

import numpy as np

import concourse.bass as bass
import concourse.tile as tile
from concourse import bacc, mybir
from concourse.bass_utils import run_bass_kernel_spmd
from concourse.masks import make_identity

B, S, E = 4, 2048, 1024
H, G, HPG, HD = 16, 4, 4, 64
KV = HPG * HD           # 256
SH = S // 2             # 1024 query rows per core
VX = HPG * (HD + 1)     # 260: V_ext row length (64 V cols + 1 ones col per head)
FP = mybir.dt.float32
AF = mybir.ActivationFunctionType
ALU = mybir.AluOpType
FPR = mybir.dt.float32r


def _r(ap):
    return ap.bitcast(FPR)

_CACHE = {}


def _body(tc, io):
    nc = tc.nc
    xb, Wq, Wk, Wv, Wo = io["xb"], io["Wq"], io["Wk"], io["Wv"], io["Wo"]
    bq, bk, bv, bo, out = io["bq"], io["bk"], io["bv"], io["bo"], io["out"]

    from contextlib import ExitStack

    with ExitStack() as es:
        const = es.enter_context(tc.tile_pool(name="const", bufs=1))
        ident = const.tile([128, 128], FP, tag="ident", name="ident")
        make_identity(nc, ident)
        ones = const.tile([128, 512], FP, tag="ones", name="ones")
        ones_st = const.tile([128, 512], FP, tag="ones_st", name="ones_st")
        nc.gpsimd.memset(ones_st, 1.0)
        nc.vector.tensor_copy(_r(ones), ones_st)
        bq_sb = const.tile([1, E], FP, tag="bq", name="bq")
        nc.sync.dma_start(_r(bq_sb), _r(bq))
        bk_sb = const.tile([1, KV], FP, tag="bk", name="bk")
        nc.sync.dma_start(_r(bk_sb), _r(bk))
        bo_sb = const.tile([1, E], FP, tag="bo", name="bo")
        nc.sync.dma_start(_r(bo_sb), _r(bo))
        # bv_ext: V bias per head + constant 1.0 in each head's ones slot.
        bvx = const.tile([1, VX], FP, tag="bvx", name="bvx")
        bvx_st = const.tile([1, VX], FP, tag="bvx_st", name="bvx_st")
        nc.gpsimd.memset(bvx_st, 1.0)
        for h in range(HPG):
            nc.sync.dma_start(bvx_st[0:1, h * 65 : h * 65 + 64], bv[0:1, h * 64 : (h + 1) * 64])
        nc.vector.tensor_copy(_r(bvx), bvx_st)

        # Persist across projection + attention phases.
        pers = es.enter_context(tc.tile_pool(name="pers", bufs=1))
        qt_sb = [pers.tile([128, SH], FP, tag=f"qt{i}", name=f"qt{i}") for i in range(8)]
        kt_dup = [pers.tile([128, S], FP, tag=f"ktd{g}", name=f"ktd{g}") for g in range(G)]
        vx_sb = [pers.tile([128, VX], FP, tag=f"vx{st}", name=f"vx{st}") for st in range(16)]

        def psum_copy(use_scalar, dst, src):
            # The Activation queue is busy issuing weight DMAs early in phase
            # A; route PSUM->SBUF copies to ScalarE only once those are done
            # (the Q-projection section), else DVE. Destinations are written
            # as fp32r so downstream matmuls see rounded operands.
            if use_scalar:
                nc.scalar.copy(_r(dst), src)
            else:
                nc.vector.tensor_copy(_r(dst), src)

        # ---------------- Phase A+B: transpose x, projections ----------------
        with tc.tile_pool(name="xtp", bufs=1) as xtp:
            xT = [xtp.tile([128, S], FP, tag=f"xT{e}", name=f"xT{e}") for e in range(8)]

            # Weights stream on the Activation DGE queue, x on the SP queue.
            with (
                tc.tile_pool(name="wqs", bufs=12) as wqs,
                tc.tile_pool(name="wks", bufs=1) as wks,
                tc.tile_pool(name="wvxs", bufs=1) as wvxs,
            ):
                wk_all = wks.tile([128, 8, KV], FP, tag="wka", name="wka")
                nc.scalar.dma_start(
                    _r(wk_all), _r(Wk.rearrange("(et p) c -> p et c", p=128))
                )
                # Wv streams into a short-lived tile; Wv_ext (zeros in the
                # ones slots; bias supplies the 1.0) is built from it with
                # strided copies, then the raw tile's space is released.
                wvx_sb = []
                with tc.tile_pool(name="wvp", bufs=2) as wvp:
                    wv_all = wvp.tile([128, 8, KV], FP, tag="wva", name="wva")
                    nc.scalar.dma_start(
                        wv_all, Wv.rearrange("(et p) c -> p et c", p=128)
                    )
                    for et in range(8):
                        st = wvp.tile([128, VX], FP, tag="wvst", name="wvst")
                        nc.gpsimd.memset(st, 0.0)
                        dst = st.rearrange("p (h w) -> p h w", h=4)[:, :, 0:64]
                        src = wv_all[:, et, :].rearrange("p (h w) -> p h w", h=4)
                        nc.vector.tensor_copy(dst, src)
                        t = wvxs.tile([128, VX], FP, tag=f"wvx{et}", name=f"wvx{et}")
                        nc.vector.tensor_copy(_r(t), st)
                        wvx_sb.append(t)
                wq_sb = []
                for ct in range(8):
                    for et in range(8):
                        w = wqs.tile([128, 128], FP, tag="wq", name="wq")
                        nc.scalar.dma_start(
                            _r(w),
                            _r(Wq[et * 128 : (et + 1) * 128, ct * 128 : (ct + 1) * 128]),
                        )
                        wq_sb.append(w)

                with (
                    tc.tile_pool(name="xin", bufs=6) as xin,
                    tc.tile_pool(name="trps", bufs=3, space="PSUM") as trps,
                ):
                    for sg in range(4):
                        xts = []
                        for j in range(4):
                            t = xin.tile([128, E], FP, tag="xin", name="xin")
                            st = sg * 4 + j
                            nc.sync.dma_start(t, xb[st * 128 : (st + 1) * 128, :])
                            xts.append(t)
                        for et in range(8):
                            ps = trps.tile([128, 512], FP, tag="trp", name="trp")
                            for j in range(4):
                                nc.tensor.transpose(
                                    ps[:, j * 128 : (j + 1) * 128],
                                    xts[j][:, et * 128 : (et + 1) * 128],
                                    ident,
                                )
                            psum_copy(False, xT[et][:, sg * 512 : (sg + 1) * 512], ps)

                with tc.tile_pool(name="pps", bufs=4, space="PSUM") as pps:
                    # KT[c, s] for all 2048 keys; per-group duplicated across
                    # both partition halves for row-packed QK^T.
                    for ct in range(2):
                        g0, g1 = 2 * ct, 2 * ct + 1
                        for sc in range(4):
                            ps = pps.tile([128, 512], FP, tag="pp", name="pp")
                            for et in range(8):
                                nc.tensor.matmul(
                                    ps,
                                    _r(wk_all[:, et, ct * 128 : (ct + 1) * 128]),
                                    _r(xT[et][:, sc * 512 : (sc + 1) * 512]),
                                    start=(et == 0),
                                    stop=False,
                                )
                            nc.tensor.matmul(
                                ps,
                                _r(bk_sb[0:1, ct * 128 : (ct + 1) * 128]),
                                _r(ones[0:1, 0:512]),
                                start=False,
                                stop=True,
                            )
                            psum_copy(
                                False, kt_dup[g0][0:64, sc * 512 : (sc + 1) * 512], ps[0:64, :]
                            )
                            psum_copy(
                                False, kt_dup[g1][64:128, sc * 512 : (sc + 1) * 512], ps[64:128, :]
                            )
                        nc.sync.dma_start(_r(kt_dup[g0][64:128, :]), _r(kt_dup[g0][0:64, :]))
                        nc.sync.dma_start(_r(kt_dup[g1][0:64, :]), _r(kt_dup[g1][64:128, :]))

                    # V_ext[s, (h, d|1)] = x @ Wv_ext (+ bv_ext outer ones)
                    for st in range(16):
                        ps = pps.tile([128, VX], FP, tag="pp", name="pp")
                        for et in range(8):
                            nc.tensor.matmul(
                                ps,
                                _r(xT[et][:, st * 128 : (st + 1) * 128]),
                                _r(wvx_sb[et]),
                                start=(et == 0),
                                stop=False,
                            )
                        nc.tensor.matmul(
                            ps, _r(ones[0:1, 0:128]), _r(bvx), start=False, stop=True
                        )
                        psum_copy(False, vx_sb[st], ps)

                    # QT[c, q] = Wq^T @ xT (+ bq outer ones)
                    for ct in range(8):
                        for qc in range(2):
                            ps = pps.tile([128, 512], FP, tag="pp", name="pp")
                            for et in range(8):
                                nc.tensor.matmul(
                                    ps,
                                    _r(wq_sb[ct * 8 + et]),
                                    _r(xT[et][:, qc * 512 : (qc + 1) * 512]),
                                    start=(et == 0),
                                    stop=False,
                                )
                            nc.tensor.matmul(
                                ps,
                                _r(bq_sb[0:1, ct * 128 : (ct + 1) * 128]),
                                _r(ones[0:1, 0:512]),
                                start=False,
                                stop=True,
                            )
                            psum_copy(
                                True, qt_sb[ct][:, qc * 512 : (qc + 1) * 512], ps
                            )

        # ---------------- Phase C: attention ----------------
        # ScalarE exp is the bottleneck engine; PSUM budget: 2 rotating score
        # tiles (2 banks each) + 2 persistent PV accumulators (2 banks each)
        # = 8 banks. One score tile per (kt, head) half-step.
        with (
            tc.tile_pool(name="aop", bufs=1) as aop,
            tc.tile_pool(name="wop", bufs=1) as wop,
        ):
            aoT = [aop.tile([128, SH], FP, tag=f"ao{t}", name=f"ao{t}") for t in range(8)]
            wo_sb = [wop.tile([128, E], FP, tag=f"wo{t}", name=f"wo{t}") for t in range(8)]
            for t in range(8):
                nc.sync.dma_start(_r(wo_sb[t]), _r(Wo[t * 128 : (t + 1) * 128, :]))

            with (
                tc.tile_pool(name="scp", bufs=2, space="PSUM") as scp,
                tc.tile_pool(name="pvp", bufs=2, space="PSUM") as pvp,
                tc.tile_pool(name="exps", bufs=4) as exps,
                tc.tile_pool(name="denp", bufs=1) as denp,
                tc.tile_pool(name="tmpp", bufs=1) as tmpp,
                tc.tile_pool(name="rbp", bufs=1) as rbp,
            ):
                def _pv(i, kt, exs, pvs):
                    g = i // 2
                    for x2 in range(2):
                        for qc in range(2):
                            nc.tensor.matmul(
                                pvs[x2][:, qc * 512 : (qc + 1) * 512],
                                _r(vx_sb[kt][:, g * 65 : (g + 1) * 65]),
                                _r(exs[x2][:, qc * 512 : (qc + 1) * 512]),
                                start=(kt == 0),
                                stop=(kt == 15),
                            )

                def _epilogue(i, pvs, last):
                    # Both heads of a pair share poff; last pair uses the PE
                    # (free PSUM) for the broadcast to shorten the tail.
                    h0 = 2 * i
                    poff = ((h0 // 4) % 2) * 64

                    # Denominator + unnormalized PV out of PSUM, interleaved
                    # per head so each accumulator is released ASAP for the
                    # next pair's PV.
                    dn = denp.tile([65, SH], FP, tag="dn", name="dn")
                    dnp = denp.tile([2, SH], FP, tag="dnp", name="dnp")
                    for x2 in range(2):
                        eng_copy = nc.scalar.copy if last else (
                            lambda d, s: nc.vector.tensor_copy(_r(d), s))
                        eng_copy(dn[64:65, :], pvs[x2][64:65, :])
                        nc.sync.dma_start(_r(dnp[x2 : x2 + 1, :]), _r(dn[64:65, :]))
                        h = 2 * i + x2
                        tidx = 2 * (h % 4) + (h // 4) // 2
                        if poff == 0:
                            nc.vector.tensor_copy(
                                _r(aoT[tidx][0:64, :]), pvs[x2][0:64, :]
                            )
                        else:
                            tm = tmpp.tile([64, SH], FP, tag="tm", name="tm")
                            if last:
                                nc.scalar.copy(tm, pvs[x2][0:64, :])
                            else:
                                nc.vector.tensor_copy(_r(tm), pvs[x2][0:64, :])
                            nc.sync.dma_start(_r(aoT[tidx][64:128, :]), _r(tm))
                    rc = denp.tile([2, SH], FP, tag="rc", name="rc")
                    nc.vector.reciprocal(rc, dnp)

                    for x2 in range(2):
                        h = 2 * i + x2
                        tidx = 2 * (h % 4) + (h // 4) // 2
                        if last:
                            # PE broadcast: ones outer rc (both partition 0,
                            # rounded to fp32r), into a free score-pool tile.
                            if x2 == 0:
                                rsrc = denp.tile([1, SH], FP, tag="rcr", name="rcr")
                                nc.vector.tensor_copy(_r(rsrc), rc[0:1, :])
                            else:
                                rc1 = denp.tile([1, SH], FP, tag="rc1", name="rc1")
                                nc.sync.dma_start(_r(rc1), _r(rc[1:2, :]))
                                rsrc = denp.tile([1, SH], FP, tag="rcr", name="rcr")
                                nc.vector.tensor_copy(_r(rsrc), rc1)
                            bc = scp.tile([128, SH], FP, tag="sc", name="sc")
                            for qc in range(2):
                                nc.tensor.matmul(
                                    bc[:, qc * 512 : (qc + 1) * 512],
                                    _r(ones[0:1, 0:128]),
                                    _r(rsrc[0:1, qc * 512 : (qc + 1) * 512]),
                                    start=True,
                                    stop=True,
                                )
                            nc.vector.tensor_tensor(
                                _r(aoT[tidx][poff : poff + 64, :]),
                                _r(aoT[tidx][poff : poff + 64, :]),
                                bc[poff : poff + 64, :],
                                ALU.mult,
                            )
                        else:
                            if x2 == 0:
                                rsrc = rc[0:1, :]
                            else:
                                rc1 = denp.tile([1, SH], FP, tag="rc1", name="rc1")
                                nc.sync.dma_start(_r(rc1), _r(rc[1:2, :]))
                                rsrc = rc1[0:1, :]
                            rb = rbp.tile([128, SH], FP, tag="rb", name="rb")
                            nc.gpsimd.partition_broadcast(rb, rsrc)
                            nc.vector.tensor_tensor(
                                _r(aoT[tidx][poff : poff + 64, :]),
                                _r(aoT[tidx][poff : poff + 64, :]),
                                _r(rb[poff : poff + 64, :]),
                                ALU.mult,
                            )

                # Flat (pair, kt) schedule: PV trails one kt step, including
                # across pair boundaries, so the PE never blocks the exp
                # stream at a pair transition.
                prev = None  # (i, kt, exs, pvs)
                pvs = None
                for i in range(8):
                    g = i // 2
                    pvs = [pvp.tile([65, SH], FP, tag="pv", name="pv") for _ in range(2)]
                    for kt in range(16):
                        scs = [scp.tile([128, SH], FP, tag="sc", name="sc") for _ in range(2)]
                        for x2 in range(2):
                            for qc in range(2):
                                nc.tensor.matmul(
                                    scs[x2][:, qc * 512 : (qc + 1) * 512],
                                    _r(kt_dup[g][x2 * 64 : (x2 + 1) * 64, kt * 128 : (kt + 1) * 128]),
                                    _r(qt_sb[i][x2 * 64 : (x2 + 1) * 64, qc * 512 : (qc + 1) * 512]),
                                    start=True,
                                    stop=True,
                                    tile_position=(x2 * 64, 0),
                                )
                        exs = []
                        for x2 in range(2):
                            ex = exps.tile([128, SH], FP, tag="ex", name="ex")
                            nc.scalar.activation(_r(ex), scs[x2], AF.Exp, scale=0.125)
                            exs.append(ex)
                        if prev is not None:
                            pi, pkt, pexs, ppvs = prev
                            _pv(pi, pkt, pexs, ppvs)
                            if pkt == 15:
                                _epilogue(pi, ppvs, last=False)
                        prev = (i, kt, exs, pvs)
                _pv(*prev)
                _epilogue(7, pvs, last=True)

            # ---------------- Phase D: O-projection ----------------
            with (
                tc.tile_pool(name="ops", bufs=4, space="PSUM") as ops,
                tc.tile_pool(name="osb", bufs=2) as osb,
            ):
                for qt in range(8):
                    ot = osb.tile([128, E], FP, tag="ot", name="ot")
                    for oc in range(2):
                        ps = ops.tile([128, 512], FP, tag="op", name="op")
                        for ct in range(8):
                            nc.tensor.matmul(
                                ps,
                                _r(aoT[ct][:, qt * 128 : (qt + 1) * 128]),
                                _r(wo_sb[ct][:, oc * 512 : (oc + 1) * 512]),
                                start=(ct == 0),
                                stop=False,
                            )
                        nc.tensor.matmul(
                            ps,
                            _r(ones[0:1, 0:128]),
                            _r(bo_sb[0:1, oc * 512 : (oc + 1) * 512]),
                            start=False,
                            stop=True,
                        )
                        psum_copy(oc == 1, ot[:, oc * 512 : (oc + 1) * 512], ps)
                    nc.sync.dma_start(out[qt * 128 : (qt + 1) * 128, :], ot)


def _build():
    if "nc" in _CACHE:
        return _CACHE["nc"]
    nc = bacc.Bacc(
        "TRN2", target_bir_lowering=False, debug=False, num_devices=8
    )
    io = {}
    io["xb"] = nc.dram_tensor("xb", [S, E], FP, kind="ExternalInput").ap()
    io["Wq"] = nc.dram_tensor("Wq", [E, E], FP, kind="ExternalInput").ap()
    io["Wk"] = nc.dram_tensor("Wk", [E, KV], FP, kind="ExternalInput").ap()
    io["Wv"] = nc.dram_tensor("Wv", [E, KV], FP, kind="ExternalInput").ap()
    io["Wo"] = nc.dram_tensor("Wo", [E, E], FP, kind="ExternalInput").ap()
    io["bq"] = nc.dram_tensor("bq", [1, E], FP, kind="ExternalInput").ap()
    io["bk"] = nc.dram_tensor("bk", [1, KV], FP, kind="ExternalInput").ap()
    io["bv"] = nc.dram_tensor("bv", [1, KV], FP, kind="ExternalInput").ap()
    io["bo"] = nc.dram_tensor("bo", [1, E], FP, kind="ExternalInput").ap()
    io["out"] = nc.dram_tensor("out", [SH, E], FP, kind="ExternalOutput").ap()
    with tile.TileContext(nc) as tc:
        _body(tc, io)
    nc.compile()
    _CACHE["nc"] = nc
    return nc


def _run(inputs, trace=False):
    x = np.ascontiguousarray(np.asarray(inputs["x"], dtype=np.float32))
    w = {k: np.ascontiguousarray(np.asarray(inputs[k], dtype=np.float32)) for k in
         ("Wq", "Wk", "Wv", "Wo")}
    bias = {k: np.ascontiguousarray(
        np.asarray(inputs[k], dtype=np.float32).reshape(1, -1)) for k in
        ("bq", "bk", "bv", "bo")}

    nc = _build()
    in_maps = []
    for b in range(B):
        for hf in range(2):
            if hf == 0:
                xbv = x[b]
            else:
                xbv = np.ascontiguousarray(
                    np.concatenate([x[b, SH:], x[b, :SH]], axis=0)
                )
            m = {"xb": xbv}
            m.update(w)
            m.update(bias)
            in_maps.append(m)

    res = run_bass_kernel_spmd(nc, in_maps, list(range(8)), trace=trace)
    out = np.empty((B, S, E), dtype=np.float32)
    for b in range(B):
        for hf in range(2):
            out[b, hf * SH : (hf + 1) * SH] = res.results[b * 2 + hf]["out"]
    return out, res


def kernel(**inputs):
    out, _ = _run(inputs, trace=False)
    return out
